# revision 1
# baseline (speedup 1.0000x reference)
"""Trainium2 Bass kernel for nn_AttentionSHA (dense transformer attention block).

Full inputs -> full output. Internally: tensor-parallel over heads across 8
NeuronCores (core g owns kv-head g and query heads 4g..4g+3; wo row-sharded),
host-side reduce of the 8 partial output projections.

Math notes (validated against the reference in fp64/fp32 numpy):
  - The reference adds a 0/1 causal mask *before* softmax (no -inf masking) and
    runs softmax over the full MAXSEQ=2048 cache axis where positions >= S hold
    zero k/v. Softmax without max-subtraction is exact here (scores are in
    [-17, 18]), so:  out = sum_t exp(sc_t)*m_t*v_t / (sum_t exp(sc_t)*m_t + 1024)
    with m_t = e if visible else 1, and +1024 = (MAXSEQ - S) zero-score tail.
    The e-factor for fully-visible regions folds into the Exp bias
    (exp(x + 1) = e*exp(x)); only the 128x128 diagonal blocks need a mask mult.
  - RoPE is applied via host-permuted weight rows (even channels then odd), a
    partition-half swap, and two multiply-adds against [cos;cos] / [-sin;sin].
"""
import numpy as np
from contextlib import ExitStack

S = 1024
D = 4096
NH = 32
NKV = 8
HD = 128
NREP = NH // NKV          # 4
MAXSEQ = 2048
NCORES = 8
DT = D // 128             # 32 d-tiles
TT = S // 128             # 8 t-tiles

_CACHE = {}


def _build_nc(phases=4, repeat=1):
    import concourse.bacc as bacc
    import concourse.mybir as mybir
    import concourse.tile as tile

    f32 = mybir.dt.float32
    f32r = mybir.dt.float32r
    Exp = mybir.ActivationFunctionType.Exp
    mult = mybir.AluOpType.mult
    add = mybir.AluOpType.add

    nc = bacc.Bacc("TRN2", target_bir_lowering=False, debug=False,
                   num_devices=NCORES)

    xT = nc.dram_tensor("xT", [D, S], f32r, kind="ExternalInput")
    wq_t = nc.dram_tensor("wq_t", [NREP, 128, DT * HD], f32r, kind="ExternalInput")
    wk_t = nc.dram_tensor("wk_t", [128, DT * HD], f32r, kind="ExternalInput")
    wv_t = nc.dram_tensor("wv_t", [128, DT * HD], f32r, kind="ExternalInput")
    wo_t = nc.dram_tensor("wo_t", [NREP * HD, D], f32r, kind="ExternalInput")
    cc_d = nc.dram_tensor("cc", [HD, S], f32, kind="ExternalInput")
    ns_d = nc.dram_tensor("ns", [HD, S], f32, kind="ExternalInput")
    emaskd_d = nc.dram_tensor("emaskd", [128, TT * 128], f32, kind="ExternalInput")
    ones_d = nc.dram_tensor("ones", [128, 128], f32r, kind="ExternalInput")
    ident_d = nc.dram_tensor("ident", [128, 128], f32, kind="ExternalInput")
    outT = nc.dram_tensor("outT", [D, S], f32, kind="ExternalOutput")

    with tile.TileContext(nc) as tc, ExitStack() as ctx:
        const = ctx.enter_context(tc.tile_pool(name="const", bufs=1))
        wts = ctx.enter_context(tc.tile_pool(name="wts", bufs=6))
        xpool = ctx.enter_context(tc.tile_pool(name="xpool", bufs=6))
        rpool = ctx.enter_context(tc.tile_pool(name="rpool", bufs=3))
        qkv = ctx.enter_context(tc.tile_pool(name="qkv", bufs=1))
        hs = ctx.enter_context(tc.tile_pool(name="hs", bufs=4))
        epool = ctx.enter_context(tc.tile_pool(name="epool", bufs=5))
        zpool = ctx.enter_context(tc.tile_pool(name="zpool", bufs=1))
        opool = ctx.enter_context(tc.tile_pool(name="opool", bufs=3))
        ps = ctx.enter_context(tc.tile_pool(name="ps", bufs=8, space="PSUM"))

        def _body():
            # ---- constants (loaded lazily at first use site) ----
            cc_sb = const.tile([128, S], f32)
            ns_sb = const.tile([128, S], f32)
            ones_sb = const.tile([128, 128], f32r)
            ident_sb = const.tile([128, 128], f32)
            emaskd_sb = const.tile([128, TT * 128], f32)

            # ---- weights; wo reuses these slots later ----
            # chunk DMAs are emitted inside the d-loop so x tiles interleave
            wq_sb = [wts.tile([128, D], f32r, name=f"wq_sb{h}", tag="w16")
                     for h in range(NREP)]
            wk_sb = wts.tile([128, D], f32r, tag="w16")
            wv_sb = wts.tile([128, D], f32r, tag="w16")

            WCHUNKS = [(d, 4) for d in range(0, DT, 4)]
            _wb = {d0: (d0, ln) for d0, ln in WCHUNKS}

            def load_w_chunk_span(d0, ln):
                c0, c1 = 128 * d0, 128 * (d0 + ln)
                for h in range(NREP):
                    nc.sync.dma_start(wq_sb[h][:, c0:c1], wq_t[h][:, c0:c1])
                nc.sync.dma_start(wk_sb[:, c0:c1], wk_t[:, c0:c1])
                nc.sync.dma_start(wv_sb[:, c0:c1], wv_t[:, c0:c1])

            if phases < 1:
                nul = const.tile([128, S], f32, name="nul")
                nc.sync.dma_start(nul[:], xT[0:128, :].bitcast(f32))
                nc.sync.dma_start(outT[0:128, :], nul[:])
                return
            # ---- phase 1: QKV projections + RoPE ----
            q_rot = [hs.tile([128, S], f32r, name=f"q_rot{h}", tag="hs")
                     for h in range(NREP)]                      # per head [e, s]
            k_rot = qkv.tile([128, S], f32r)                    # [e, t]
            v_et = qkv.tile([128, S], f32)                      # [e, t] pre-transpose
            v_te = qkv.tile([128, TT * 128], f32r)              # tile t: [t-part, e]

            for sh in range(2):
                s0 = 512 * sh
                q_ps = [ps.tile([128, 512], f32, tag="ps", name=f"q_ps{sh}_{h}")
                        for h in range(NREP)]
                k_ps = ps.tile([128, 512], f32, tag="ps", name=f"k_ps{sh}")
                v_ps = ps.tile([128, 512], f32, tag="ps", name=f"v_ps{sh}")
                for d in range(DT):
                    x_r = xpool.tile([128, 512], f32r, name="x_r")
                    nc.sync.dma_start(x_r[:], xT[128 * d:128 * (d + 1), s0:s0 + 512])
                    if sh == 0 and d in _wb:
                        load_w_chunk_span(*_wb[d])
                    for h in range(NREP):
                        nc.tensor.matmul(q_ps[h][:], wq_sb[h][:, 128 * d:128 * (d + 1)],
                                         x_r[:], start=(d == 0), stop=(d == DT - 1))
                    nc.tensor.matmul(k_ps[:], wk_sb[:, 128 * d:128 * (d + 1)],
                                     x_r[:], start=(d == 0), stop=(d == DT - 1))
                    nc.tensor.matmul(v_ps[:], wv_sb[:, 128 * d:128 * (d + 1)],
                                     x_r[:], start=(d == 0), stop=(d == DT - 1))

                if sh == 0:
                    nc.sync.dma_start(cc_sb[:], cc_d[:])
                    nc.sync.dma_start(ns_sb[:], ns_d[:])

                # RoPE: dest = psum*[cos;cos] + swap(psum)*[-sin;sin].
                # fast=True splits the swap copies across ACT+DVE — used for
                # q0 and k, whose rope latency gates phase 3's first scores
                def rope(psum, dest, fast=False):
                    sw = rpool.tile([128, 512], f32, name="sw")
                    if fast:
                        nc.vector.tensor_copy(sw[0:64, :], psum[64:128, :])
                    else:
                        nc.scalar.copy(sw[0:64, :], psum[64:128, :])
                    nc.scalar.copy(sw[64:128, :], psum[0:64, :])
                    t1 = rpool.tile([128, 512], f32, name="t1")
                    nc.vector.tensor_tensor(t1[:], psum[:], cc_sb[:, s0:s0 + 512], op=mult)
                    t2 = rpool.tile([128, 512], f32, name="t2")
                    nc.gpsimd.tensor_tensor(t2[:], sw[:], ns_sb[:, s0:s0 + 512], op=mult)
                    nc.vector.tensor_tensor(dest, t1[:], t2[:], op=add)

                nc.vector.tensor_copy(v_et[:, s0:s0 + 512], v_ps[:])
                rope(q_ps[0], q_rot[0][:, s0:s0 + 512], fast=(sh == 1))
                rope(k_ps, k_rot[:, s0:s0 + 512], fast=(sh == 1))
                for h in range(1, NREP):
                    rope(q_ps[h], q_rot[h][:, s0:s0 + 512], fast=(sh == 1))
                if phases >= 2 and (sh == 0 or phases == 2):
                    if sh == 0:
                        nc.sync.dma_start(ident_sb[:], ident_d[:])
                    for t in range(4 * sh, 4 * (sh + 1)):
                        tr = ps.tile([128, 128], f32, tag="ps", name="tr")
                        nc.tensor.transpose(tr[:], v_et[:, 128 * t:128 * (t + 1)],
                                            ident_sb[:])
                        nc.vector.tensor_copy(v_te[:, 128 * t:128 * (t + 1)], tr[:])

            if phases < 2:
                nc.sync.dma_start(outT[0:128, :], v_et[:])

            if phases == 2:
                nc.sync.dma_start(outT[0:128, :], v_te[:].bitcast(f32))
            # ---- phase 3: attention per head ----
            att = []                                  # per head [e, s], normalized
            inv_sqrt_hd = float(1.0 / np.sqrt(HD))
            if phases >= 3:
                nc.sync.dma_start(ones_sb[:], ones_d[:])
                nc.sync.dma_start(emaskd_sb[:], emaskd_d[:])
                # s-half-1 V transposes, deferred from phase 1: not consumed
                # until head 0's t=4 PV matmul, so they overlap the first
                # scores/exp instead of blocking phase 3 behind the rope queue
                for t in range(4, TT):
                    tr = ps.tile([128, 128], f32, tag="ps", name="tr")
                    nc.tensor.transpose(tr[:], v_et[:, 128 * t:128 * (t + 1)],
                                        ident_sb[:])
                    nc.scalar.copy(v_te[:, 128 * t:128 * (t + 1)], tr[:])
            for h in range(NREP if phases >= 3 else 0):
                z_ps = [ps.tile([128, 512], f32, tag="ps", name=f"z_ps{h}_{c}")
                        for c in range(2)]
                o_ps = [ps.tile([128, 512], f32, tag="ps", name=f"o_ps{h}_{c}")
                        for c in range(2)]
                def emit_sc_exp(t):
                    dlo, dhi = 128 * t, 128 * (t + 1)
                    expm = epool.tile([128, S], f32r, name="expm")
                    for c in range(2):
                        sc = ps.tile([128, 512], f32, tag="ps", name="sc")
                        nc.tensor.matmul(sc[:], k_rot[:, dlo:dhi],
                                         q_rot[h][:, 512 * c:512 * (c + 1)],
                                         start=True, stop=True)
                        lo, hi = 512 * c, 512 * (c + 1)
                        if dlo >= hi:
                            # fully invisible: plain exp
                            nc.scalar.activation(expm[:, lo:hi], sc[:], Exp,
                                                 scale=inv_sqrt_hd)
                        elif dhi <= lo:
                            # fully visible: exp(x + 1) = e * exp(x)
                            nc.scalar.activation(expm[:, lo:hi], sc[:], Exp,
                                                 scale=inv_sqrt_hd, bias=1.0)
                        else:
                            # diagonal block inside this chunk: one exp call,
                            # then the mask factors applied in-place (diag x
                            # emaskd on GpSimd; visible remainder x e on DVE)
                            nc.scalar.activation(expm[:, lo:hi], sc[:], Exp,
                                                 scale=inv_sqrt_hd)
                            nc.gpsimd.tensor_tensor(
                                expm[:, dlo:dhi], expm[:, dlo:dhi],
                                emaskd_sb[:, 128 * t:128 * (t + 1)], op=mult)
                            if dhi < hi:
                                nc.gpsimd.tensor_scalar_mul(
                                    expm[:, dhi:hi], expm[:, dhi:hi],
                                    float(np.e))
                    return expm

                pend = [emit_sc_exp(0), emit_sc_exp(1)]
                for t in range(TT):
                    if t + 2 < TT:
                        pend.append(emit_sc_exp(t + 2))
                    expm_t = pend.pop(0)
                    # z pair then o pair: the stationary operand (ones / v_te
                    # tile) is reused by consecutive matmuls, and the two psum
                    # groups still alternate within each pair
                    for c in range(2):
                        nc.tensor.matmul(z_ps[c][:], ones_sb[:],
                                         expm_t[:, 512 * c:512 * (c + 1)],
                                         start=(t == 0), stop=(t == TT - 1))
                    for c in range(2):
                        nc.tensor.matmul(o_ps[c][:], v_te[:, 128 * t:128 * (t + 1)],
                                         expm_t[:, 512 * c:512 * (c + 1)],
                                         start=(t == 0), stop=(t == TT - 1))
                z_sb = zpool.tile([128, S], f32, name="z_sb")
                rz = zpool.tile([128, S], f32, name="rz")
                a = hs.tile([128, S], f32r, name=f"att{h}", tag="hs")
                for c in range(2):
                    nc.vector.tensor_scalar_add(z_sb[:, 512 * c:512 * (c + 1)],
                                                z_ps[c][:], float(MAXSEQ - S))
                    nc.vector.reciprocal(rz[:, 512 * c:512 * (c + 1)],
                                         z_sb[:, 512 * c:512 * (c + 1)])
                    nc.vector.tensor_tensor(a[:, 512 * c:512 * (c + 1)],
                                            o_ps[c][:], rz[:, 512 * c:512 * (c + 1)],
                                            op=mult)
                att.append(a)

            if phases == 3:
                for h in range(NREP):
                    nc.sync.dma_start(outT[128 * h:128 * (h + 1), :], att[h][:].bitcast(f32))
            # ---- phase 4: output projection (partial over this core's 512 cols) ----
            wo_sb = []
            for h in range(NREP if phases >= 4 else 0):
                w = wts.tile([128, D], f32r, name=f"wo_sb{h}", tag="w16")
                nc.sync.dma_start(w[:], wo_t[128 * h:128 * (h + 1), :])
                wo_sb.append(w)

            for do in range(DT if phases >= 4 else 0):
                op_ps = [ps.tile([128, 512], f32, tag="ps", name=f"op{c}")
                         for c in range(2)]
                for h in range(NREP):
                    for c in range(2):
                        nc.tensor.matmul(op_ps[c][:],
                                         wo_sb[h][:, 128 * do:128 * (do + 1)],
                                         att[h][:, 512 * c:512 * (c + 1)],
                                         start=(h == 0), stop=(h == NREP - 1))
                out_sb = opool.tile([128, S], f32, name="out_sb")
                nc.vector.tensor_copy(out_sb[:, 0:512], op_ps[0][:])
                nc.scalar.copy(out_sb[:, 512:1024], op_ps[1][:])
                nc.sync.dma_start(outT[128 * do:128 * (do + 1), :], out_sb[:])


        for _rep in range(repeat):
            _body()

    nc.compile()
    return nc


def _to_f32r(x):
    """Host replica of the device fp32 -> fp32r conversion: round-to-nearest-
    even to an 11-bit mantissa (low 12 bits zeroed). Verified bit-exact against
    the DVE/DMA converters."""
    xi = np.ascontiguousarray(x, np.float32).view(np.uint32).astype(np.uint64)
    r = ((xi + 0x7FF + ((xi >> 12) & 1)) >> 12) << 12
    return (r & 0xFFFFFFFF).astype(np.uint32).view(np.float32)


def kernel(**inputs):
    from concourse.bass_utils import run_bass_kernel_spmd

    x = np.asarray(inputs["x"], np.float32)                 # [1, S, D]
    cos = np.asarray(inputs["freqs_cos"], np.float32)       # [S, 64]
    sin = np.asarray(inputs["freqs_sin"], np.float32)       # [S, 64]
    wq = np.asarray(inputs["wq"], np.float32)               # [NH, HD, D]
    wk = np.asarray(inputs["wk"], np.float32)               # [NKV, HD, D]
    wv = np.asarray(inputs["wv"], np.float32)               # [NKV, HD, D]
    wo = np.asarray(inputs["wo"], np.float32)               # [D, D]
    input_pos = np.asarray(inputs["input_pos"]).astype(np.int64)  # [S]

    if "nc" not in _CACHE:
        _CACHE["nc"] = _build_nc()
    nc = _CACHE["nc"]

    perm = np.concatenate([np.arange(0, HD, 2), np.arange(1, HD, 2)])
    xT = _to_f32r(x[0].T)                                   # [D, S] fp32r-encoded
    cc = np.ascontiguousarray(np.concatenate([cos.T, cos.T], 0))   # [128, S]
    ns = np.ascontiguousarray(np.concatenate([-sin.T, sin.T], 0))  # [128, S]
    # visibility adds +1 pre-exp where input_pos[t] <= input_pos[s]; for the
    # (spec-guaranteed) sorted arange fill only diagonal blocks are mixed.
    emaskd_t = np.empty((TT, 128, 128), np.float32)
    for t in range(TT):
        p = input_pos[128 * t:128 * (t + 1)]
        emaskd_t[t] = np.where(p[:, None] <= p[None, :], np.float32(np.e),
                               np.float32(1.0))
    # partition-major [128, TT*128] so the single DMA reads 4KB runs
    emaskd = np.ascontiguousarray(
        emaskd_t.transpose(1, 0, 2).reshape(128, TT * 128))
    ones128 = np.ones((128, 128), np.float32)
    ident = np.eye(128, dtype=np.float32)

    in_maps = []
    for g in range(NCORES):
        wq_g = wq[NREP * g:NREP * (g + 1)][:, perm, :]       # [4, 128, D]

        def pmajor(wT):
            # [D, 128e] -> [128p, DT*128e]: partition-major so each chunk DMA
            # reads 2KB-contiguous runs per partition
            return np.ascontiguousarray(
                wT.reshape(DT, 128, HD).transpose(1, 0, 2).reshape(128, DT * HD))

        in_maps.append({
            "xT": xT,
            "wq_t": _to_f32r(np.stack([pmajor(wq_g[j].T) for j in range(NREP)])),
            "wk_t": _to_f32r(pmajor(wk[g][perm].T)),     # [128, DT*128]
            "wv_t": _to_f32r(pmajor(wv[g].T)),           # [128, DT*128]
            "wo_t": _to_f32r(
                wo[:, NREP * HD * g:NREP * HD * (g + 1)].T),         # [512, D]
            "cc": cc, "ns": ns, "emaskd": emaskd,
            "ones": _to_f32r(ones128), "ident": ident,
        })

    res = run_bass_kernel_spmd(nc, in_maps, list(range(NCORES)))
    total = np.zeros((D, S), np.float64)
    for g in range(NCORES):
        total += res.results[g]["outT"]
    return np.ascontiguousarray(total.T.astype(np.float32)[None])   # [1, S, D]



# revision 5
# speedup vs baseline: 1.1625x; 1.1625x over previous
"""Trainium2 Bass kernel for nn_AttentionSHA (dense transformer attention block).

Full inputs -> full output. Internally: tensor-parallel over heads across 8
NeuronCores (core g owns kv-head g and query heads 4g..4g+3; wo row-sharded),
host-side reduce of the 8 partial output projections (bf16 partials).

Performance: the QKV and WO projections run as fp8(e4m3) DoubleRow matmuls
(0.5 PE cycles per output row, 256-deep contraction per instruction) using a
hi+lo residual split of both operands for accuracy:
    a*b ~= a_hi*b_hi + (a_hi*b_lo + a_lo*b_hi)      [lo*lo dropped]
where v_hi = fp8(v), v_lo = fp8(v - v_hi) share one scale so all three
products accumulate in a single PSUM group. Per 128-deep contraction tile
this is 1.5 DoubleRow instructions (384 cycles/tile-pair * ... = 0.75x fp32r)
and recovers ~10-bit effective mantissa (measured bit-exact vs ml_dtypes
emulation on device). The attention core (scores / exp / z / PV) stays fp32r.

Math notes (validated against the reference in fp64/fp32 numpy):
  - The reference adds a 0/1 causal mask *before* softmax (no -inf masking) and
    runs softmax over the full MAXSEQ=2048 cache axis where positions >= S hold
    zero k/v. Softmax without max-subtraction is exact here (scores are in
    [-17, 18]), so:  out = sum_t exp(sc_t)*m_t*v_t / (sum_t exp(sc_t)*m_t + 1024)
    with m_t = e if visible else 1, and +1024 = (MAXSEQ - S) zero-score tail.
    The e-factor for fully-visible regions folds into the Exp bias
    (exp(x + 1) = e*exp(x)); only the 128x128 diagonal blocks need a mask mult.
  - RoPE is applied via host-permuted weight rows (even channels then odd), a
    partition-half swap, and two multiply-adds against [cos;cos] / [-sin;sin].
  - Scales: wq/wk/wv stored *64 (cos/sin tables pre-divided by 64 unscale q,k
    in the rope); v flows *64 into PV, fixed by z' = (z+1024)*4 so the
    normalized att comes out *16 = the fp8 storage scale for the WO moving
    operand; wo stored *64; final psum *1024 -> output copy scales by 2^-10.
"""
import numpy as np
from contextlib import ExitStack

S = 1024
D = 4096
NH = 32
NKV = 8
HD = 128
NREP = NH // NKV          # 4
MAXSEQ = 2048
NCORES = 8
DT = D // 128             # 32 d-tiles
TT = S // 128             # 8 t-tiles
NP = DT // 2              # 16 d-tile pairs

SW = 64.0                 # weight fp8 scale (wq/wk/wv/wo)
S8 = 16.0                 # att fp8 scale

_CACHE = {}


def _build_nc():
    import concourse.bacc as bacc
    import concourse.mybir as mybir
    import concourse.tile as tile

    f32 = mybir.dt.float32
    f32r = mybir.dt.float32r
    bf16 = mybir.dt.bfloat16
    f8 = mybir.dt.float8e4
    Exp = mybir.ActivationFunctionType.Exp
    Copy = mybir.ActivationFunctionType.Copy
    mult = mybir.AluOpType.mult
    add = mybir.AluOpType.add
    sub = mybir.AluOpType.subtract
    DR = mybir.MatmulPerfMode.DoubleRow

    nc = bacc.Bacc("TRN2", target_bir_lowering=False, debug=False,
                   num_devices=NCORES)

    # x hi/lo interleaved, chunk-contiguous: [128, pair m(16), sh(2), sub(4), 512]
    # sub order per pair: (xl_2m, xh_2m, xl_2m+1, xh_2m+1)
    xc_d = nc.dram_tensor("xc", [128, NP * 2 * 4 * 512], f8, kind="ExternalInput")
    # weights hi/lo interleaved: [128, 64 sub, 128]; sub order (wh_0, wl_0, ...)
    wq_d = nc.dram_tensor("wq_c", [NREP, 128, 64 * HD], f8, kind="ExternalInput")
    wk_d = nc.dram_tensor("wk_c", [128, 64 * HD], f8, kind="ExternalInput")
    wv_d = nc.dram_tensor("wv_c", [128, 64 * HD], f8, kind="ExternalInput")
    # wo hi/lo interleaved over the head axis: [128, 8 sub, D]
    wo_d = nc.dram_tensor("wo_c", [128, 8 * D], f8, kind="ExternalInput")
    cc_d = nc.dram_tensor("cc", [HD, S], f32, kind="ExternalInput")
    ns_d = nc.dram_tensor("ns", [HD, S], f32, kind="ExternalInput")
    emaskd_d = nc.dram_tensor("emaskd", [128, TT * 128], f32, kind="ExternalInput")
    ones_d = nc.dram_tensor("ones", [128, 128], f32r, kind="ExternalInput")
    ident_d = nc.dram_tensor("ident", [128, 128], f32, kind="ExternalInput")
    outT = nc.dram_tensor("outT", [D, S], bf16, kind="ExternalOutput")

    with tile.TileContext(nc) as tc, ExitStack() as ctx:
        const = ctx.enter_context(tc.tile_pool(name="const", bufs=1))
        wts = ctx.enter_context(tc.tile_pool(name="wts", bufs=6))
        xpool = ctx.enter_context(tc.tile_pool(name="xpool", bufs=4))
        rpool = ctx.enter_context(tc.tile_pool(name="rpool", bufs=3))
        qkv = ctx.enter_context(tc.tile_pool(name="qkv", bufs=1))
        hs = ctx.enter_context(tc.tile_pool(name="hs", bufs=4))
        epool = ctx.enter_context(tc.tile_pool(name="epool", bufs=5))
        zpool = ctx.enter_context(tc.tile_pool(name="zpool", bufs=1))
        apool = ctx.enter_context(tc.tile_pool(name="apool", bufs=2))
        opool = ctx.enter_context(tc.tile_pool(name="opool", bufs=3))
        ps = ctx.enter_context(tc.tile_pool(name="ps", bufs=8, space="PSUM"))

        def _body():
            # ---- constants (loaded lazily at first use site) ----
            cc_sb = const.tile([128, S], f32)
            ns_sb = const.tile([128, S], f32)
            ones_sb = const.tile([128, 128], f32r)
            ident_sb = const.tile([128, 128], f32)
            emaskd_sb = const.tile([128, TT * 128], f32)

            # ---- weights; wo reuses these slots later ----
            wq_sb = [wts.tile([128, 64, HD], f8, name=f"wq_sb{h}", tag="w16")
                     for h in range(NREP)]
            wk_sb = wts.tile([128, 64, HD], f8, tag="w16")
            wv_sb = wts.tile([128, 64, HD], f8, tag="w16")

            def load_w_span(m0, mn):
                # subtile span for d-pairs m0..m0+mn
                c0, c1 = 4 * m0 * HD, 4 * (m0 + mn) * HD
                s0, s1 = 4 * m0, 4 * (m0 + mn)
                for h in range(NREP):
                    nc.sync.dma_start(
                        wq_sb[h][:, s0:s1, :],
                        wq_d[h][:, c0:c1].rearrange("p (s f) -> p s f", f=HD))
                nc.sync.dma_start(
                    wk_sb[:, s0:s1, :],
                    wk_d[:, c0:c1].rearrange("p (s f) -> p s f", f=HD))
                nc.sync.dma_start(
                    wv_sb[:, s0:s1, :],
                    wv_d[:, c0:c1].rearrange("p (s f) -> p s f", f=HD))

            _wb = {m0: (m0, 4) for m0 in range(0, NP, 4)}

            # ---- phase 1: QKV projections (fp8 DoubleRow) + RoPE ----
            q_rot = [hs.tile([128, S], f32r, name=f"q_rot{h}", tag="hs")
                     for h in range(NREP)]                      # per head [e, s]
            k_rot = qkv.tile([128, S], f32r)                    # [e, t]
            v_et = qkv.tile([128, S], f32)                      # [e, t] pre-transpose
            v_te = qkv.tile([128, TT * 128], f32r)              # tile t: [t-part, e]

            for sh in range(2):
                s0 = 512 * sh
                q_ps = [ps.tile([128, 512], f32, tag="ps", name=f"q_ps{sh}_{h}")
                        for h in range(NREP)]
                k_ps = ps.tile([128, 512], f32, tag="ps", name=f"k_ps{sh}")
                v_ps = ps.tile([128, 512], f32, tag="ps", name=f"v_ps{sh}")
                for m in range(NP):
                    x_r = xpool.tile([128, 4, 512], f8, name="x_r")
                    xoff = (m * 2 + sh) * 4 * 512
                    nc.sync.dma_start(
                        x_r[:],
                        xc_d[:, xoff:xoff + 4 * 512].rearrange(
                            "p (s f) -> p s f", f=512))
                    if sh == 0 and m in _wb:
                        load_w_span(*_wb[m])
                    st = (m == 0)
                    sp = False
                    for dst, w in ([(q_ps[h], wq_sb[h]) for h in range(NREP)]
                                   + [(k_ps, wk_sb), (v_ps, wv_sb)]):
                        # hi*hi for both d-tiles of the pair
                        nc.tensor.matmul(dst[:], w[:, 4 * m:4 * m + 3:2, :],
                                         x_r[:, 1:4:2, :],
                                         start=st, stop=sp, perf_mode=DR)
                        # per d-tile: w_hi*x_lo + w_lo*x_hi
                        nc.tensor.matmul(dst[:], w[:, 4 * m:4 * m + 2, :],
                                         x_r[:, 0:2, :],
                                         start=False, stop=False, perf_mode=DR)
                        nc.tensor.matmul(dst[:], w[:, 4 * m + 2:4 * m + 4, :],
                                         x_r[:, 2:4, :],
                                         start=False, stop=(m == NP - 1),
                                         perf_mode=DR)

                if sh == 0:
                    nc.sync.dma_start(cc_sb[:], cc_d[:])
                    nc.sync.dma_start(ns_sb[:], ns_d[:])

                # RoPE: dest = psum*[cos;cos] + swap(psum)*[-sin;sin].
                # fast=True splits the swap copies across ACT+DVE — used for
                # q0 and k, whose rope latency gates phase 3's first scores
                def rope(psum, dest, fast=False):
                    sw = rpool.tile([128, 512], f32, name="sw")
                    if fast:
                        nc.vector.tensor_copy(sw[0:64, :], psum[64:128, :])
                    else:
                        nc.scalar.copy(sw[0:64, :], psum[64:128, :])
                    nc.scalar.copy(sw[64:128, :], psum[0:64, :])
                    t1 = rpool.tile([128, 512], f32, name="t1")
                    nc.vector.tensor_tensor(t1[:], psum[:], cc_sb[:, s0:s0 + 512], op=mult)
                    t2 = rpool.tile([128, 512], f32, name="t2")
                    nc.gpsimd.tensor_tensor(t2[:], sw[:], ns_sb[:, s0:s0 + 512], op=mult)
                    nc.vector.tensor_tensor(dest, t1[:], t2[:], op=add)

                nc.vector.tensor_copy(v_et[:, s0:s0 + 512], v_ps[:])
                rope(q_ps[0], q_rot[0][:, s0:s0 + 512], fast=(sh == 1))
                rope(k_ps, k_rot[:, s0:s0 + 512], fast=(sh == 1))
                for h in range(1, NREP):
                    rope(q_ps[h], q_rot[h][:, s0:s0 + 512], fast=(sh == 1))
                if sh == 0:
                    nc.sync.dma_start(ident_sb[:], ident_d[:])
                    for t in range(4):
                        tr = ps.tile([128, 128], f32, tag="ps", name="tr")
                        nc.tensor.transpose(tr[:], v_et[:, 128 * t:128 * (t + 1)],
                                            ident_sb[:])
                        nc.vector.tensor_copy(v_te[:, 128 * t:128 * (t + 1)], tr[:])

            # ---- phase 3: attention per head ----
            # att (normalized, *S8) stored fp8 hi/lo interleaved (lo first)
            att_c = apool.tile([128, 8, S], f8, name="att_c", tag="attc", bufs=1)
            inv_sqrt_hd = float(1.0 / np.sqrt(HD))
            nc.sync.dma_start(ones_sb[:], ones_d[:])
            nc.sync.dma_start(emaskd_sb[:], emaskd_d[:])
            # s-half-1 V transposes, deferred from phase 1: not consumed
            # until head 0's t=4 PV matmul, so they overlap the first
            # scores/exp instead of blocking phase 3 behind the rope queue
            for t in range(4, TT):
                tr = ps.tile([128, 128], f32, tag="ps", name="tr")
                nc.tensor.transpose(tr[:], v_et[:, 128 * t:128 * (t + 1)],
                                    ident_sb[:])
                nc.scalar.copy(v_te[:, 128 * t:128 * (t + 1)], tr[:])
            for h in range(NREP):
                z_ps = [ps.tile([128, 512], f32, tag="ps", name=f"z_ps{h}_{c}")
                        for c in range(2)]
                o_ps = [ps.tile([128, 512], f32, tag="ps", name=f"o_ps{h}_{c}")
                        for c in range(2)]
                def emit_sc_exp(t):
                    dlo, dhi = 128 * t, 128 * (t + 1)
                    expm = epool.tile([128, S], f32r, name="expm")
                    for c in range(2):
                        sc = ps.tile([128, 512], f32, tag="ps", name="sc")
                        nc.tensor.matmul(sc[:], k_rot[:, dlo:dhi],
                                         q_rot[h][:, 512 * c:512 * (c + 1)],
                                         start=True, stop=True)
                        lo, hi = 512 * c, 512 * (c + 1)
                        if dlo >= hi:
                            # fully invisible: plain exp
                            nc.scalar.activation(expm[:, lo:hi], sc[:], Exp,
                                                 scale=inv_sqrt_hd)
                        elif dhi <= lo:
                            # fully visible: exp(x + 1) = e * exp(x)
                            nc.scalar.activation(expm[:, lo:hi], sc[:], Exp,
                                                 scale=inv_sqrt_hd, bias=1.0)
                        else:
                            # diagonal block inside this chunk: one exp call,
                            # then the mask factors applied in-place (diag x
                            # emaskd on GpSimd; visible remainder x e on DVE)
                            nc.scalar.activation(expm[:, lo:hi], sc[:], Exp,
                                                 scale=inv_sqrt_hd)
                            nc.gpsimd.tensor_tensor(
                                expm[:, dlo:dhi], expm[:, dlo:dhi],
                                emaskd_sb[:, 128 * t:128 * (t + 1)], op=mult)
                            if dhi < hi:
                                nc.gpsimd.tensor_scalar_mul(
                                    expm[:, dhi:hi], expm[:, dhi:hi],
                                    float(np.e))
                    return expm

                pend = [emit_sc_exp(0), emit_sc_exp(1)]
                for t in range(TT):
                    if t + 2 < TT:
                        pend.append(emit_sc_exp(t + 2))
                    expm_t = pend.pop(0)
                    # z pair then o pair: the stationary operand (ones / v_te
                    # tile) is reused by consecutive matmuls, and the two psum
                    # groups still alternate within each pair
                    for c in range(2):
                        nc.tensor.matmul(z_ps[c][:], ones_sb[:],
                                         expm_t[:, 512 * c:512 * (c + 1)],
                                         start=(t == 0), stop=(t == TT - 1))
                    for c in range(2):
                        nc.tensor.matmul(o_ps[c][:], v_te[:, 128 * t:128 * (t + 1)],
                                         expm_t[:, 512 * c:512 * (c + 1)],
                                         start=(t == 0), stop=(t == TT - 1))
                z_sb = zpool.tile([128, S], f32, name="z_sb")
                rz = zpool.tile([128, S], f32, name="rz")
                at = apool.tile([128, S], f32, name="at", tag="at")
                for c in range(2):
                    cs = slice(512 * c, 512 * (c + 1))
                    # z' = (z + tail) * (SW/S8): v is *SW, att target *S8
                    nc.vector.tensor_scalar(z_sb[:, cs], z_ps[c][:],
                                            float(MAXSEQ - S), SW / S8,
                                            op0=add, op1=mult)
                    nc.vector.reciprocal(rz[:, cs], z_sb[:, cs])
                    nc.vector.tensor_tensor(at[:, cs], o_ps[c][:], rz[:, cs],
                                            op=mult)
                    # hi = fp8(at) on ACT; lo = at - hi on GpSimd
                    nc.scalar.activation(att_c[:, 2 * h + 1, cs], at[:, cs], Copy)
                    nc.gpsimd.tensor_tensor(att_c[:, 2 * h, cs], at[:, cs],
                                            att_c[:, 2 * h + 1, cs], op=sub)

            # ---- phase 4: output projection (fp8 DoubleRow over this core's
            # 512 att channels; psum is *S8*SW = 1024x, output copy scales back)
            wo_sb = wts.tile([128, 8, D], f8, name="wo_sb", tag="wo", bufs=1)
            nc.sync.dma_start(wo_sb[:],
                              wo_d[:].rearrange("p (s f) -> p s f", f=D))

            for do in range(DT):
                dc = slice(128 * do, 128 * (do + 1))
                op_ps = [ps.tile([128, 512], f32, tag="ps", name=f"op{c}")
                         for c in range(2)]
                for c in range(2):
                    cs = slice(512 * c, 512 * (c + 1))
                    # hi*hi for head pairs (0,1) and (2,3)
                    nc.tensor.matmul(op_ps[c][:], wo_sb[:, 0:3:2, dc],
                                     att_c[:, 1:4:2, cs],
                                     start=True, stop=False, perf_mode=DR)
                    nc.tensor.matmul(op_ps[c][:], wo_sb[:, 4:7:2, dc],
                                     att_c[:, 5:8:2, cs],
                                     start=False, stop=False, perf_mode=DR)
                    # per head: wo_hi*att_lo + wo_lo*att_hi
                    for hh in range(NREP):
                        nc.tensor.matmul(op_ps[c][:],
                                         wo_sb[:, 2 * hh:2 * hh + 2, dc],
                                         att_c[:, 2 * hh:2 * hh + 2, cs],
                                         start=False, stop=(hh == NREP - 1),
                                         perf_mode=DR)
                out_sb = opool.tile([128, S], bf16, name="out_sb")
                nc.vector.tensor_scalar_mul(out_sb[:, 0:512], op_ps[0][:],
                                            1.0 / (S8 * SW))
                nc.scalar.activation(out_sb[:, 512:1024], op_ps[1][:], Copy,
                                     scale=1.0 / (S8 * SW))
                nc.sync.dma_start(outT[dc, :], out_sb[:])

        _body()

    nc.compile()
    return nc


def _to_f32r(x):
    """Host replica of the device fp32 -> fp32r conversion: round-to-nearest-
    even to an 11-bit mantissa (low 12 bits zeroed). Verified bit-exact against
    the DVE/DMA converters."""
    xi = np.ascontiguousarray(x, np.float32).view(np.uint32).astype(np.uint64)
    r = ((xi + 0x7FF + ((xi >> 12) & 1)) >> 12) << 12
    return (r & 0xFFFFFFFF).astype(np.uint32).view(np.float32)


def _split8(x):
    """fp8(e4m3) hi + lo residual split, shared scale. Returns (hi, lo)."""
    import ml_dtypes
    E4 = ml_dtypes.float8_e4m3
    hi = np.asarray(x, np.float32).astype(E4)
    lo = (np.asarray(x, np.float32) - hi.astype(np.float32)).astype(E4)
    return hi, lo


def kernel(**inputs):
    from concourse.bass_utils import run_bass_kernel_spmd

    x = np.asarray(inputs["x"], np.float32)                 # [1, S, D]
    cos = np.asarray(inputs["freqs_cos"], np.float32)       # [S, 64]
    sin = np.asarray(inputs["freqs_sin"], np.float32)       # [S, 64]
    wq = np.asarray(inputs["wq"], np.float32)               # [NH, HD, D]
    wk = np.asarray(inputs["wk"], np.float32)               # [NKV, HD, D]
    wv = np.asarray(inputs["wv"], np.float32)               # [NKV, HD, D]
    wo = np.asarray(inputs["wo"], np.float32)               # [D, D]
    input_pos = np.asarray(inputs["input_pos"]).astype(np.int64)  # [S]

    if "nc" not in _CACHE:
        _CACHE["nc"] = _build_nc()
    nc = _CACHE["nc"]

    perm = np.concatenate([np.arange(0, HD, 2), np.arange(1, HD, 2)])

    # x: [D, S] -> [128p, pair m, sh, (xl_2m, xh_2m, xl_2m+1, xh_2m+1), 512]
    xT = x[0].T                                             # [D, S]
    xh, xl = _split8(xT)
    x4 = np.empty((DT, 2, 128, S), dtype=xh.dtype)          # [d-tile, lo/hi, p, s]
    x4[:, 1] = xh.reshape(DT, 128, S)
    x4[:, 0] = xl.reshape(DT, 128, S)
    # -> [p, m, sh, sub(4), 512]: sub = (d-pair member, lo/hi) interleaved
    xc = np.ascontiguousarray(
        x4.reshape(NP, 4, 128, 2, 512)                      # [m, sub, p, sh, 512]
        .transpose(2, 0, 3, 1, 4)                           # [p, m, sh, sub, 512]
        .reshape(128, NP * 2 * 4 * 512))

    def wsplit(wT):
        # [D, 128e] -> hi/lo interleaved [128p, 64 sub, 128e]
        h8, l8 = _split8(wT * SW)
        out = np.empty((128, 64, HD), dtype=h8.dtype)
        out[:, 0::2, :] = h8.reshape(DT, 128, HD).transpose(1, 0, 2)
        out[:, 1::2, :] = l8.reshape(DT, 128, HD).transpose(1, 0, 2)
        return out.reshape(128, 64 * HD)

    # cos/sin tables absorb the 1/SW unscale of q,k
    cc = np.ascontiguousarray(np.concatenate([cos.T, cos.T], 0)) / SW   # [128, S]
    ns = np.ascontiguousarray(np.concatenate([-sin.T, sin.T], 0)) / SW  # [128, S]
    # visibility adds +1 pre-exp where input_pos[t] <= input_pos[s]; for the
    # (spec-guaranteed) sorted arange fill only diagonal blocks are mixed.
    emaskd_t = np.empty((TT, 128, 128), np.float32)
    for t in range(TT):
        p = input_pos[128 * t:128 * (t + 1)]
        emaskd_t[t] = np.where(p[:, None] <= p[None, :], np.float32(np.e),
                               np.float32(1.0))
    # partition-major [128, TT*128] so the single DMA reads 4KB runs
    emaskd = np.ascontiguousarray(
        emaskd_t.transpose(1, 0, 2).reshape(128, TT * 128))
    ones128 = np.ones((128, 128), np.float32)
    ident = np.eye(128, dtype=np.float32)

    in_maps = []
    for g in range(NCORES):
        wq_g = wq[NREP * g:NREP * (g + 1)][:, perm, :]       # [4, 128, D]

        # wo rows for this core's 4 heads, hi/lo interleaved [128, 8 sub, D]
        wo_g = wo[:, NREP * HD * g:NREP * HD * (g + 1)].T    # [512, D]
        wh8, wl8 = _split8(wo_g * SW)
        wo_c = np.empty((128, 8, D), dtype=wh8.dtype)
        wo_c[:, 0::2, :] = wh8.reshape(NREP, 128, D).transpose(1, 0, 2)
        wo_c[:, 1::2, :] = wl8.reshape(NREP, 128, D).transpose(1, 0, 2)

        in_maps.append({
            "xc": xc,
            "wq_c": np.stack([wsplit(wq_g[j].T) for j in range(NREP)]),
            "wk_c": wsplit(wk[g][perm].T),
            "wv_c": wsplit(wv[g].T),
            "wo_c": wo_c.reshape(128, 8 * D),
            "cc": cc, "ns": ns, "emaskd": emaskd,
            "ones": _to_f32r(ones128), "ident": ident,
        })

    res = run_bass_kernel_spmd(nc, in_maps, list(range(NCORES)))
    total = np.zeros((D, S), np.float32)
    for g in range(NCORES):
        total += res.results[g]["outT"].astype(np.float32)
    return np.ascontiguousarray(total.T[None])   # [1, S, D]
